# revision 23
# baseline (speedup 1.0000x reference)
"""Trainium2 Bass kernel for Transformer-XL style attention (nn_Attention2).

Reference computation per (b, h):
    rw_q = q + r_w_bias[h];  rr_q = q + r_bias[h]          # [S, D]
    scores = (rw_q @ k^T + rr_q @ r^T) / sqrt(D)           # [S, S]
    scores = where(causal, scores, -1e9)
    p = softmax(scores, -1)                                # output 1 [S, S]
    out = p @ v                                            # output 2 [S, D]

B=2, S=2048, H=16, D=64  ->  32 independent (b, h) pairs, 4 per NeuronCore.

Device strategy per (b, h) pair (all-bf16 matmuls, f32 accumulation):
  - host supplies qT/kT/rT in [D, S] layout; k and r stacked into one
    [128, S] tile so AC and BD run as two concurrent K=64 row-tiled
    matmuls accumulating into the same PSUM bank.
  - per 128-query row block: scores for columns [0, (qblk+1)*128) only
    (causal lower triangle; the untouched upper triangle relies on
    pre-zeroed ExternalOutput buffers).
  - ONE ACT exp pass (scale=1/8) PSUM->SBUF bf16, with accum_out giving
    the row sums for free.  No max-subtraction (scores are O(10), exp is
    safe in f32, and masked entries are exactly 0 after exp(-1e9/8)).
  - rs = 1/rowsum (DVE).  p_norm = p_u * rs on GpSimd (f32 out) -> HBM.
  - p_u is block-transposed with one xbar DMA-transpose per row block;
    PV matmuls (stationary = p^T block, moving = v block) accumulate
    out[q, d] in PSUM; final scale by rs on DVE -> HBM.
"""

import sys

import numpy as np

for _p in ("/opt/trn_rl_repo", "/root/.axon_site/_ro/trn_rl_repo"):
    if _p not in sys.path:
        sys.path.append(_p)

B, S, H, D = 2, 2048, 16, 64
N_CORES = 8
PAIRS = (B * H) // N_CORES  # 4 (b,h) pairs per core
QBLK = 128                  # query rows per block
NBLK = S // QBLK            # 16 row blocks
PSUM_CHUNK = 1024           # PSUM score region width (2 banks)
MM_N = 512                  # max moving free dim per fp-matmul
MASK_NEG = -1e9


def split_multi_waits(nc):
    """walrus (this snapshot) encodes at most ONE semaphore wait per
    instruction (NEURON_ISA_TPB_EVENTS has a single wait slot) and errors
    with 'Too many sync wait commands' otherwise.  Tile occasionally emits
    instructions with several on_wait conditions (e.g. the kernel-tail
    drain, or a consumer whose inputs arrived via different DMA queues).
    Split the extras into standalone EventSemaphore waits on the same
    engine, immediately before the instruction — semantics are identical
    (the engine stream blocks on each in turn)."""
    from concourse import mybir

    # find an unused semaphore id range (Tile assigns ids without going
    # through nc's allocator, so alloc_semaphore would collide)
    max_id = 0
    for fn in nc.m.functions:
        for blk in fn.blocks:
            for inst in blk.instructions:
                si = inst.sync_info
                if si is None:
                    continue
                for w in list(si.on_wait) + list(si.on_update):
                    if w.id is not None:
                        max_id = max(max_id, w.id)
    dummies = {}

    def dummy_for(engine):
        if engine not in dummies:
            dummies[engine] = (
                max_id + 1 + len(dummies),
                f"presplit_dummy_{engine.name}",
            )
        return dummies[engine]

    n_split = 0
    for fn in nc.m.functions:
        for blk in fn.blocks:
            insts = blk.instructions
            out = []
            changed = False
            for inst in insts:
                si = inst.sync_info
                if si is not None and si.on_wait and len(si.on_wait) > 1:
                    waits = list(si.on_wait)
                    for w in waits[:-1]:
                        ev = mybir.InstEventSemaphore(
                            name=f"{inst.name}-presplit{n_split}", ins=[], outs=[]
                        )
                        ev.engine = inst.engine
                        # EVSEM wants an update leg; +0 on a dedicated unused
                        # per-engine semaphore is a no-op (DMA queue sems are
                        # off-limits, shared sems trip the race detector)
                        dummy_id, dummy_name = dummy_for(inst.engine)
                        noop = mybir.SyncUpdate(
                            sync_type="semaphore", id=dummy_id,
                            ant_name=dummy_name,
                            update_mode="sem-add-imm", update_value=0,
                        )
                        ev.sync_info = mybir.SyncInfo(on_wait=[w], on_update=[noop])
                        out.append(ev)
                        n_split += 1
                    inst.sync_info = mybir.SyncInfo(
                        on_wait=[waits[-1]], on_update=list(si.on_update)
                    )
                    changed = True
                out.append(inst)
            if changed:
                blk.instructions = out
    return n_split


def build_nc(pairs: int = PAIRS, nblk: int = NBLK, do_transpose: bool = True, do_pv: bool = True, bufs_ptu: int = 3, bufs_pn: int = 3, bufs_sps: int = 3, chunk: int = PSUM_CHUNK):
    """Build the per-core Bass program (SPMD: same program on all cores)."""
    from contextlib import ExitStack

    import concourse.bass as bass
    import concourse.tile as tile
    from concourse import mybir
    from concourse.masks import make_lower_triangular

    f32 = mybir.dt.float32
    bf16 = mybir.dt.bfloat16
    Exp = mybir.ActivationFunctionType.Exp

    nc = bass.Bass(num_swdge_queues=4)
    qt = nc.declare_dram_parameter("qt", [pairs, 2 * D, S], bf16, isOutput=False)
    krt = nc.declare_dram_parameter("krt", [pairs, 2 * D, S], bf16, isOutput=False)
    vv = nc.declare_dram_parameter("vv", [pairs, S, D], bf16, isOutput=False)
    bias2 = nc.declare_dram_parameter("bias2", [2 * D, pairs], f32, isOutput=False)
    p_out = nc.declare_dram_parameter("p_out", [pairs, S, S], f32, isOutput=True)
    o_out = nc.declare_dram_parameter("o_out", [pairs, S, D], f32, isOutput=True)

    with ExitStack() as ctx:
        tc = ctx.enter_context(tile.TileContext(nc))

        consts = ctx.enter_context(tc.tile_pool(name="consts", bufs=1))
        qk_pool = ctx.enter_context(tc.tile_pool(name="qk", bufs=2))
        v_pool = ctx.enter_context(tc.tile_pool(name="v", bufs=2))
        ptu_pool = ctx.enter_context(tc.tile_pool(name="ptu", bufs=bufs_ptu))
        pt_pool = ctx.enter_context(tc.tile_pool(name="pt", bufs=2))
        pn_pool = ctx.enter_context(tc.tile_pool(name="pn", bufs=bufs_pn))
        small = ctx.enter_context(tc.tile_pool(name="small", bufs=8))
        sps_pool = ctx.enter_context(tc.tile_pool(name="sps", bufs=bufs_sps, space="PSUM"))
        pv_pool = ctx.enter_context(tc.tile_pool(name="pv", bufs=2, space="PSUM"))

        # lower-triangular (incl diag) 0/1 multiplicative causal mask for the
        # diagonal block, applied on SBUF after exp (DVE must not touch PSUM:
        # PE-write over a DVE-read bank on pool reuse hangs real hardware)
        tri01 = consts.tile([QBLK, QBLK], bf16)
        make_lower_triangular(nc, tri01[:], val=1.0, diag=True)

        bias_sb = consts.tile([2 * D, pairs], f32)
        nc.gpsimd.dma_start(out=bias_sb[:], in_=bias2[:])

        for p in range(pairs):
            # [0:64] = q^T (for AC against k^T), [64:128] = q^T (for BD against r^T)
            qb = qk_pool.tile([2 * D, S], bf16, tag="qb")
            nc.gpsimd.dma_start(out=qb[:], in_=qt[p])
            # add r_w_bias[h] to the top half, r_bias[h] to the bottom half
            # (tensor_tensor with a free-broadcast AP: the TensorScalarPtr
            # encoding runs out of sync-wait slots under Tile)
            nc.vector.tensor_add(
                qb[0:D, :], qb[0:D, :],
                bias_sb[0:D, p : p + 1].to_broadcast((D, S)),
            )
            nc.vector.tensor_add(
                qb[D : 2 * D, :], qb[D : 2 * D, :],
                bias_sb[D : 2 * D, p : p + 1].to_broadcast((D, S)),
            )

            kr = qk_pool.tile([2 * D, S], bf16, tag="kr")
            nc.gpsimd.dma_start(out=kr[:], in_=krt[p])

            v_sb = v_pool.tile([QBLK, nblk, D], bf16, tag="v")
            nc.gpsimd.dma_start(
                out=v_sb[:],
                in_=vv[p].rearrange("(n p) d -> p n d", p=QBLK)[:, 0:nblk, :],
            )

            for qi in range(nblk):
                ncols = (qi + 1) * QBLK
                q0 = qi * QBLK
                qcols = qb[:, q0 : q0 + QBLK]

                ptu = ptu_pool.tile([QBLK, S], bf16, tag="ptu")
                sums = []
                nd = ncols - QBLK  # diagonal block starts here
                for c0 in range(0, ncols, chunk):
                    w = min(chunk, ncols - c0)
                    sps = sps_pool.tile([QBLK, chunk], f32, tag="sps")
                    for cc in range(0, w, MM_N):
                        ww = min(MM_N, w - cc)
                        # one K=128 matmul: contraction over the stacked
                        # [q+rwb ; q+rbb] x [k ; r] halves gives AC+BD directly
                        nc.tensor.matmul(
                            sps[:, cc : cc + ww],
                            lhsT=qcols,
                            rhs=kr[:, c0 + cc : c0 + cc + ww],
                        )
                    # exp the non-diagonal part (with row-sum accumulator)
                    wa = min(w, nd - c0)
                    if wa > 0:
                        acc = small.tile([QBLK, 1], f32, tag="acc")
                        nc.scalar.activation(
                            out=ptu[:, c0 : c0 + wa],
                            in_=sps[:, :wa],
                            func=Exp,
                            scale=1.0 / np.sqrt(D),
                            accum_out=acc[:],
                        )
                        sums.append(acc)
                    if c0 + w == ncols:  # chunk containing the diagonal block
                        # exp without accum; mask + partial sum on SBUF (DVE)
                        nc.scalar.activation(
                            out=ptu[:, nd:ncols],
                            in_=sps[:, w - QBLK : w],
                            func=Exp,
                            scale=1.0 / np.sqrt(D),
                        )
                        nc.vector.tensor_mul(
                            ptu[:, nd:ncols], ptu[:, nd:ncols], tri01[:]
                        )
                        accd = small.tile([QBLK, 1], f32, tag="acc")
                        nc.vector.reduce_sum(
                            out=accd[:], in_=ptu[:, nd:ncols],
                            axis=mybir.AxisListType.X,
                        )
                        sums.append(accd)

                tot = sums[0]
                for extra in sums[1:]:
                    tot2 = small.tile([QBLK, 1], f32, tag="acc")
                    nc.vector.tensor_add(tot2[:], tot[:], extra[:])
                    tot = tot2
                rs = small.tile([QBLK, 1], f32, tag="rs")
                nc.vector.reciprocal(rs[:], tot[:])

                out_sb = small.tile([QBLK, D], f32, tag="osb")
                if do_transpose:
                    # blocked transpose: pt[kp, i, q] = ptu[q, i*128 + kp]
                    pt = pt_pool.tile([QBLK, nblk, QBLK], bf16, tag="pt")
                    nc.sync.dma_start_transpose(
                        out=pt[:, : qi + 1, :], in_=ptu[:, :ncols]
                    )
                if do_transpose and do_pv:
                    pv = pv_pool.tile([QBLK, D], f32, tag="pv")
                    for ki in range(qi + 1):
                        nc.tensor.matmul(
                            pv[:],
                            lhsT=pt[:, ki, :],
                            rhs=v_sb[:, ki, :],
                            start=(ki == 0),
                            stop=(ki == qi),
                        )
                    nc.vector.tensor_mul(
                        out_sb[:], pv[:], rs[:].to_broadcast((QBLK, D))
                    )
                else:
                    nc.vector.memset(out_sb[:], 0.0)
                nc.gpsimd.dma_start(out=o_out[p, q0 : q0 + QBLK, :], in_=out_sb[:])

                pn = pn_pool.tile([QBLK, S], f32, tag="pn")
                nc.vector.tensor_mul(
                    pn[:, :ncols], ptu[:, :ncols],
                    rs[:].to_broadcast((QBLK, ncols)),
                )
                nc.gpsimd.dma_start(
                    out=p_out[p, q0 : q0 + QBLK, 0:ncols], in_=pn[:, :ncols]
                )

    split_multi_waits(nc)
    return nc


def shard_inputs(query, key, value, r, r_bias, r_w_bias):
    """Host-side sharding/layout: returns in_maps for run_bass_kernel_spmd."""
    import ml_dtypes

    bf16 = ml_dtypes.bfloat16
    q = np.asarray(query, np.float32)
    k = np.asarray(key, np.float32)
    v = np.asarray(value, np.float32)
    rr = np.asarray(r, np.float32)
    rb = np.asarray(r_bias, np.float32)
    rwb = np.asarray(r_w_bias, np.float32)

    in_maps = []
    for c in range(N_CORES):
        qt = np.empty((PAIRS, 2 * D, S), bf16)
        krt = np.empty((PAIRS, 2 * D, S), bf16)
        vv = np.empty((PAIRS, S, D), bf16)
        bias2 = np.empty((2 * D, PAIRS), np.float32)
        for p in range(PAIRS):
            f = PAIRS * c + p
            b, h = divmod(f, H)
            qT = q[b, :, h, :].T.astype(bf16)
            qt[p, 0:D] = qT
            qt[p, D:] = qT
            krt[p, 0:D] = k[b, :, h, :].T.astype(bf16)
            krt[p, D:] = rr[b, :, h, :].T.astype(bf16)
            vv[p] = v[b, :, h, :].astype(bf16)
            bias2[0:D, p] = rwb[h]
            bias2[D:, p] = rb[h]
        in_maps.append({"qt": qt, "krt": krt, "vv": vv, "bias2": bias2})
    return in_maps


def assemble_outputs(results):
    out = np.empty((B, H, S, D), np.float32)
    p_attn = np.empty((B, H, S, S), np.float32)
    for c in range(N_CORES):
        for p in range(PAIRS):
            b, h = divmod(PAIRS * c + p, H)
            out[b, h] = results[c]["o_out"][p]
            p_attn[b, h] = results[c]["p_out"][p]
    return out, p_attn


_NC_CACHE = {}


def kernel(query, key, value, r, r_bias, r_w_bias, mask=None):
    from concourse import bass_utils

    if "nc" not in _NC_CACHE:
        _NC_CACHE["nc"] = build_nc()
    nc = _NC_CACHE["nc"]
    in_maps = shard_inputs(query, key, value, r, r_bias, r_w_bias)
    res = bass_utils.run_bass_kernel_spmd(nc, in_maps, core_ids=list(range(N_CORES)))
    return assemble_outputs(res.results)
